# revision 106
# baseline (speedup 1.0000x reference)
"""Multi-Head Latent Attention (MLA) forward on 8 Trainium2 NeuronCores.

Problem shapes (hardcoded, self-contained):
  B=2, T=2048, D=2048, H=16, DH=128, DKV=512, DQ=1024, DR=64, fp32 I/O.

Sharding: core ci = b*4 + hg  (b in {0,1}, hg in {0..3}); each core owns one
batch element and 4 heads.  Up-projection weights sharded over heads; the
final W_O matmul is input-dim sharded, so each core emits a partial (D,T)
output which the host sums (in f32) over the 4 head-group cores per batch.

All device matmul operands are bf16 (PSUM accumulation stays f32) except
the gathered cq latents and the Q/QR up-projection weights, which travel
and multiply as fp8-e4m3: cq only feeds the q/score path, where the
quantization noise is benign (measured 1.27e-2 end-to-end vs the 2e-2
gate), and fp8 halves the AllGather so the replicated work fully covers
it.  The host pre-converts all inputs, so there are no on-device rounding
copies.  Dataflow is feature-major (features on partitions, tokens free)
so every contraction lands on the partition dim with no transposes of
activations.

Phases (single NEFF):
  Stage 1: the cq compression (8 of 13 latent blocks) is token-sharded —
     each core computes cq only for its 512-token quarter (quarter = ci%4,
     from a dedicated x_cq input so the program stays SPMD-uniform),
     RMS-normalizes it locally, and kicks off an HBM AllGather across the
     batch group ([0-3] / [4-7]).
  Stage 2 (covers the AllGather): replicated ckv | kr latents per 512-token
     tile (W_KR packed twice in the last block so the duplicated kr halves
     come out of one matmul) + the K/V up-projections and kr RoPE.  RMS
     stats via ACT squares + DVE accumulate + one ones-matmul reduce; the
     per-token 1/rms commutes through the up-projections, so they consume
     RAW latents and the scale lands on the outputs, with the norm chain's
     tiny PE ops threaded between UK matmul groups.
  Stage 3: Q/QR up-projections for all tiles from the gathered fp8 cq
     (read back tile-by-tile through the gpsimd DMA queue, consumed
     directly by fp8 matmuls), RoPE on qr.
  C: per (head, 512-query tile): exact-width causal S^T blocks (keys on
     partitions), one static 128x128 diagonal mask, exp -> bf16 P^T; PV
     matmuls trail S/exp by five key-blocks and accumulate a ones-column
     denominator; completed query blocks normalize on DVE and PE-transpose
     one step later into feature-major aoT, so the in-order PE never waits
     on ACT or DVE.
  D: final.T = W_O_shard.T @ aoT -> DRAM (D, T) bf16, out-DMAs batched
     4 dc-blocks per transfer to respect SP-sequencer dispatch cost.
"""

import math

import numpy as np

B, T, D = 2, 2048, 2048
H, DH = 16, 128
DKV, DQ, DR = 512, 1024, 64
ROPE_BASE = 500000.0
EPS = 1e-6
SCALE = 1.0 / math.sqrt(DH + DR)

HL = 4            # heads per core
NCORES = 8
TW = 512          # token tile width for A+B
NT = T // TW      # 4 token tiles
NKC = D // 128    # 16 contraction chunks over D
MTOT = 1664       # latent columns: 1024 cq | 512 ckv | 64 kr | 64 kr (dup)
NMC = MTOT // 128  # 13 column blocks
NEG = -1.0e30

_CACHE: dict = {}
LAST_EXEC_NS = None


def _build():
    from contextlib import ExitStack

    import concourse.mybir as mybir
    import concourse.tile as tile
    from concourse.bacc import Bacc
    from concourse.masks import make_identity

    f32 = mybir.dt.float32
    bf16 = mybir.dt.bfloat16
    f8 = mybir.dt.float8e4
    AF = mybir.ActivationFunctionType

    nc = Bacc("TRN2", num_devices=8)

    xT_d = nc.dram_tensor("xt", (D, T), bf16, kind="ExternalInput")
    # x columns for this core's token quarter (quarter index = ci % 4): the
    # cq latent blocks are computed only for these tokens, then AllGathered
    xcq_d = nc.dram_tensor("xcq", (D, TW), bf16, kind="ExternalInput")
    # weights host-relaid to (128, blocks, NKC*128) so each 128-column latent
    # block is one contiguous full-bandwidth DMA
    wallq_d = nc.dram_tensor("wallq", (128, 8, D), bf16, kind="ExternalInput")
    wallkv_d = nc.dram_tensor(
        "wallkv", (128, 5, D), bf16, kind="ExternalInput")
    wuq_d = nc.dram_tensor("wuq", (DQ, HL * DH), f8, kind="ExternalInput")
    wqr_d = nc.dram_tensor("wqr", (DQ, HL * DR), f8, kind="ExternalInput")
    wuk_d = nc.dram_tensor("wuk", (DKV, HL * DH), bf16, kind="ExternalInput")
    wuv_d = nc.dram_tensor("wuv", (DKV, HL * DH), bf16, kind="ExternalInput")
    wo_d = nc.dram_tensor("wo", (HL * DH, D), bf16, kind="ExternalInput")
    cos_d = nc.dram_tensor("costab", (128, T), bf16, kind="ExternalInput")
    sin_d = nc.dram_tensor("sintab", (128, T), bf16, kind="ExternalInput")
    out_d = nc.dram_tensor("final_t", (D, T), bf16, kind="ExternalOutput")

    with tile.TileContext(nc) as tc, ExitStack() as ctx:
        persist = ctx.enter_context(tc.tile_pool(name="persist", bufs=1))

        # --- constants ---
        ones_sb = persist.tile([128, 128], bf16, tag="ones")
        nc.gpsimd.memset(ones_sb, 1.0)
        ident_bf = persist.tile([128, 128], bf16, tag="identbf")
        make_identity(nc, ident_bf)
        eps_sb = persist.tile([1, 1], f32, tag="eps")
        nc.vector.memset(eps_sb, EPS)
        # signed-permutation matrix for rotate_half: P[s, d] = -1 where
        # s = d+32, +1 where s = d-32 (per 64-row rope half), so that
        # matmul(lhsT=P, rhs=v) computes rotate_half(v) on the PE
        pmat_sb = persist.tile([128, 128], bf16, tag="pmat")
        nc.gpsimd.memset(pmat_sb, 0.0)
        for hh in range(2):
            b0 = hh * 64
            for (r0, c0, fill) in ((b0 + 32, b0, -1.0), (b0, b0 + 32, 1.0)):
                sub = pmat_sb[r0:r0 + 32, c0:c0 + 32]
                nc.gpsimd.affine_select(
                    out=sub, in_=sub,
                    compare_op=mybir.AluOpType.not_equal, fill=fill,
                    base=0, pattern=[[1, 32]], channel_multiplier=-1)

        # --- persistent activations (consumed by phase C/D) ---
        qT_sb = persist.tile([128, HL, T], bf16, tag="qT")
        qrT_sb = persist.tile([128, HL // 2, T], bf16, tag="qrT")
        kT_sb = persist.tile([128, HL, T], bf16, tag="kT")
        v_sb = persist.tile([128, HL, T // 128, 129], bf16, tag="v")
        nc.vector.memset(v_sb[:, :, :, 128:129], 1.0)
        kr_rope = persist.tile([128, T], bf16, tag="kr_rope")

        def rope(dst, src, tmp1, rot_sb, cos_ap, sin_ap):
            # dst = src*cos + rotate_half(src)*sin over two 64-row halves
            for hh in range(2):
                lo = slice(hh * 64, hh * 64 + 32)
                hi = slice(hh * 64 + 32, hh * 64 + 64)
                nc.scalar.mul(rot_sb[lo, :], src[hi, :], -1.0)
                nc.scalar.copy(rot_sb[hi, :], src[lo, :])
            nc.vector.tensor_mul(tmp1, src, cos_ap)
            nc.vector.tensor_mul(rot_sb, rot_sb, sin_ap)
            nc.vector.tensor_add(dst, tmp1, rot_sb)

        dramp = ctx.enter_context(tc.tile_pool(name="dram", bufs=1,
                                               space="DRAM"))
        cq_shard = dramp.tile([DQ, TW], f8, tag="cqShard")
        cq_gath = dramp.tile([NT * DQ, TW], f8, tag="cqGath")

        # ===== Stage 1: cq latents for this core's token quarter only, =====
        # normalized locally, then AllGathered across the batch group while
        # stage 2's replicated work covers the collective latency.  One pool
        # scope for all stages (no pool-close barrier between them); stage-1
        # tiles share tags with their stage-2/3 counterparts.
        with tc.tile_pool(name="wKV", bufs=1) as wKV, \
             tc.tile_pool(name="wB", bufs=1) as wB, \
             tc.tile_pool(name="trig", bufs=1) as trig, \
             tc.tile_pool(name="xA", bufs=2) as xA, \
             tc.tile_pool(name="cqP", bufs=3) as cqP, \
             tc.tile_pool(name="s1cq", bufs=1) as s1cq, \
             tc.tile_pool(name="ckvP", bufs=1) as ckvP, \
             tc.tile_pool(name="krP", bufs=1) as krP, \
             tc.tile_pool(name="sqA", bufs=1) as sqA, \
             tc.tile_pool(name="nrmA", bufs=1) as nrmA, \
             tc.tile_pool(name="psMM", bufs=5, space="PSUM") as psMM, \
             tc.tile_pool(name="psSum", bufs=1, space="PSUM") as psSum, \
             tc.tile_pool(name="psT", bufs=1, space="PSUM") as psT, \
             tc.tile_pool(name="psBC", bufs=1, space="PSUM") as psBC:

            # stage-1 inputs first (collective kickoff is the critical path),
            # then tile-0 x / wallkv, then the B weights and trig tables
            xq_sb = xA.tile([128, NKC, TW], bf16, tag="xA")
            wallq_sb = wKV.tile([128, 8, D], bf16, tag="wallq")
            nc.sync.dma_start(out=xq_sb[:, 0, :], in_=xcq_d[0:128, :])
            nc.sync.dma_start(out=wallq_sb[:, 0, :], in_=wallq_d[:, 0, :])
            next_mc = 1
            for kc in range(1, NKC):
                nc.sync.dma_start(
                    out=xq_sb[:, kc, :], in_=xcq_d[kc * 128:(kc + 1) * 128, :])
                if kc % 4 == 0 and next_mc < 8:
                    nc.sync.dma_start(
                        out=wallq_sb[:, next_mc, :], in_=wallq_d[:, next_mc, :])
                    next_mc += 1
            for mc in range(next_mc, 8):
                nc.sync.dma_start(
                    out=wallq_sb[:, mc, :], in_=wallq_d[:, mc, :])
            x0_sb = xA.tile([128, NKC, TW], bf16, tag="xA")
            wallkv_sb = wKV.tile([128, 5, D], bf16, tag="wallkv")
            for kc in range(NKC):
                nc.sync.dma_start(
                    out=x0_sb[:, kc, :], in_=xT_d[kc * 128:(kc + 1) * 128, 0:TW])
            for mc in range(5):
                nc.sync.dma_start(
                    out=wallkv_sb[:, mc, :], in_=wallkv_d[:, mc, :])
            wuq_sb = wB.tile([128, DQ // 128, HL * DH], f8, tag="wuq")
            wqr_sb = wB.tile([128, DQ // 128, HL * DR], f8, tag="wqr")
            wuk_sb = wB.tile([128, DKV // 128, HL * DH], bf16, tag="wuk")
            wuv_sb = wB.tile([128, DKV // 128, HL * DH], bf16, tag="wuv")
            nc.sync.dma_start(
                out=wuk_sb, in_=wuk_d.rearrange("(c p) m -> p c m", p=128))
            nc.sync.dma_start(
                out=wuv_sb, in_=wuv_d.rearrange("(c p) m -> p c m", p=128))
            cos_sb = trig.tile([128, T], bf16, tag="cos")
            sin_sb = trig.tile([128, T], bf16, tag="sin")
            nc.sync.dma_start(out=cos_sb, in_=cos_d[:, :])
            nc.sync.dma_start(out=sin_sb, in_=sin_d[:, :])

            # ---- stage-1 latent groups (this core's token quarter) ----
            cq1_sb = s1cq.tile([128, DQ // 128, TW], bf16, tag="cq")
            cq8_out = cqP.tile([128, DQ // 128, TW], f8, tag="cq8")
            acc_cq = sqA.tile([128, TW], f32, tag="accCq")
            acc_cq_bf = sqA.tile([128, TW], bf16, tag="accCqBf")
            for mc in range(8):
                mm_ps = psMM.tile([128, TW], f32, tag="psMM")
                for kc in range(NKC):
                    nc.tensor.matmul(
                        mm_ps,
                        lhsT=wallq_sb[:, mc, kc * 128:(kc + 1) * 128],
                        rhs=xq_sb[:, kc, :],
                        start=(kc == 0), stop=(kc == NKC - 1))
                nc.scalar.copy(cq1_sb[:, mc, :], mm_ps)
                if mc == 0:
                    nc.scalar.square(acc_cq, mm_ps)
                else:
                    sq_sb = sqA.tile([128, TW], bf16, tag="sqA")
                    nc.scalar.square(sq_sb, mm_ps)
                    out = acc_cq_bf if mc == 7 else acc_cq
                    nc.vector.tensor_add(out, acc_cq, sq_sb)

            # stage-1 norm chain threaded between tile-0's first A-kv groups:
            # the fp8 collective has ~25us of covering slack, so a slightly
            # later kickoff is free while the PE stall on the chain is not
            s1_state = {}

            def s1_finish_part1():
                sum_ps = psSum.tile([1, TW], f32, tag="psSum")
                nc.tensor.matmul(
                    sum_ps, lhsT=ones_sb[:, 0:1], rhs=acc_cq_bf,
                    start=True, stop=True)
                nrm_f = acc_cq[0:1, :]
                nc.scalar.activation(
                    nrm_f, sum_ps, func=AF.Sqrt, bias=eps_sb, scale=1.0 / DQ)
                nc.vector.reciprocal(nrm_f, nrm_f)
                nrm_q = nrmA.tile([1, TW], bf16, tag="nrmBf")
                nc.vector.tensor_copy(nrm_q, nrm_f)
                s1_state["nrm_q"] = nrm_q

            def s1_finish_part2():
                bc_ps = psBC.tile([128, TW], f32, tag="psBC")
                nc.tensor.matmul(
                    bc_ps, lhsT=ones_sb[0:1, :], rhs=s1_state["nrm_q"],
                    start=True, stop=True)
                bc_q = nrmA.tile([128, TW], bf16, tag="bcSb")
                nc.vector.tensor_copy(bc_q, bc_ps)
                for mc in range(8):
                    nc.vector.tensor_mul(
                        cq8_out[:, mc, :], cq1_sb[:, mc, :], bc_q)
                nc.gpsimd.dma_start(
                    out=cq_shard[:, :].rearrange("(c p) t -> p c t", p=128),
                    in_=cq8_out)
                nc.gpsimd.collective_compute(
                    "AllGather", mybir.AluOpType.bypass,
                    replica_groups=[[0, 1, 2, 3], [4, 5, 6, 7]],
                    ins=[cq_shard.opt()], outs=[cq_gath.opt()])

            for tt in range(NT):
                ts = slice(tt * TW, (tt + 1) * TW)
                if tt == 0:
                    x_sb = x0_sb
                else:
                    x_sb = xA.tile([128, NKC, TW], bf16, tag="xA")
                    for kc in range(NKC):
                        nc.sync.dma_start(
                            out=x_sb[:, kc, :],
                            in_=xT_d[kc * 128:(kc + 1) * 128, ts])

                ckv_sb = ckvP.tile([128, DKV // 128, TW], bf16, tag="ckv")
                kr_sb = krP.tile([128, TW], bf16, tag="kr")
                acc_kv = sqA.tile([128, TW], f32, tag="accKv")
                acc_kv_bf = sqA.tile([128, TW], bf16, tag="accKvBf")
                for mc in range(5):
                    mm_ps = psMM.tile([128, TW], f32, tag="psMM")
                    for kc in range(NKC):
                        nc.tensor.matmul(
                            mm_ps,
                            lhsT=wallkv_sb[:, mc, kc * 128:(kc + 1) * 128],
                            rhs=x_sb[:, kc, :],
                            start=(kc == 0), stop=(kc == NKC - 1))
                    dst = ckv_sb[:, mc, :] if mc < 4 else kr_sb
                    nc.scalar.copy(dst, mm_ps)
                    if mc == 0:
                        nc.scalar.square(acc_kv, mm_ps)
                    elif mc < 4:
                        sq_sb = sqA.tile([128, TW], bf16, tag="sqA")
                        nc.scalar.square(sq_sb, mm_ps)
                        out = acc_kv_bf if mc == 3 else acc_kv
                        nc.vector.tensor_add(out, acc_kv, sq_sb)
                    if tt == 0 and mc == 0:
                        s1_finish_part1()
                    elif tt == 0 and mc == 1:
                        s1_finish_part2()

                # kv norm reduce + sqrt chain immediately: the add(3) is done
                # by the end of the kr group, and the sqrt beats the rope's
                # scalar ops into the ACT queue so it lands promptly
                sum_kv = psSum.tile([1, TW], f32, tag="psSum")
                nc.tensor.matmul(
                    sum_kv, lhsT=ones_sb[:, 0:1], rhs=acc_kv_bf,
                    start=True, stop=True)
                nrm_f = acc_kv[0:1, :]
                nc.scalar.activation(
                    nrm_f, sum_kv, func=AF.Sqrt, bias=eps_sb, scale=1.0 / DKV)
                nc.vector.reciprocal(nrm_f, nrm_f)
                nrm_kv = nrmA.tile([1, TW], bf16, tag="nrmBf")
                nc.vector.tensor_copy(nrm_kv, nrm_f)
                # kr rope early (keeps the tile's DVE tail short)
                tmp1b = sqA.tile([128, TW], bf16, tag="sqA")
                rot_b = sqA.tile([128, TW], bf16, tag="accKvBf")
                rope(kr_rope[:, ts], kr_sb, tmp1b, rot_b,
                     cos_sb[:, ts], sin_sb[:, ts])

                # B-KV with the kv-norm chain's PE ops threaded between
                # the UK groups (post-scale on outputs)
                def uk_group(h):
                    mm_ps = psMM.tile([128, TW], f32, tag="psMM")
                    for kc in range(DKV // 128):
                        nc.tensor.matmul(
                            mm_ps,
                            lhsT=wuk_sb[:, kc, h * 128:(h + 1) * 128],
                            rhs=ckv_sb[:, kc, :],
                            start=(kc == 0), stop=(kc == DKV // 128 - 1))
                    return mm_ps

                uk_ps = [uk_group(0), uk_group(1)]
                bc_ps = psBC.tile([128, TW], f32, tag="psBC")
                nc.tensor.matmul(
                    bc_ps, lhsT=ones_sb[0:1, :], rhs=nrm_kv,
                    start=True, stop=True)
                t_ps = psT.tile([128, 4], f32, tag="psT")
                for tc4 in range(TW // 128):
                    nc.tensor.matmul(
                        t_ps[:, tc4:tc4 + 1],
                        lhsT=nrm_kv[0:1, tc4 * 128:(tc4 + 1) * 128],
                        rhs=ones_sb[0:1, 0:1],
                        start=True, stop=True)
                uk_ps += [uk_group(2), uk_group(3)]
                bc_kv = nrmA.tile([128, TW], bf16, tag="bcSb")
                nc.vector.tensor_copy(bc_kv, bc_ps)
                bcT = {}
                for tc4 in range(TW // 128):
                    tv = nrmA.tile([128, 1], f32, tag=f"bcT{tc4}")
                    nc.vector.tensor_copy(tv, t_ps[:, tc4:tc4 + 1])
                    bcT[tc4] = tv
                for h in range(HL):
                    nc.vector.tensor_mul(kT_sb[:, h, ts], uk_ps[h], bc_kv)
                for tc4 in range(TW // 128):
                    mm_ps = psMM.tile([128, TW], f32, tag="psMM")
                    for kc in range(DKV // 128):
                        nc.tensor.matmul(
                            mm_ps,
                            lhsT=ckv_sb[:, kc, tc4 * 128:(tc4 + 1) * 128],
                            rhs=wuv_sb[:, kc, :],
                            start=(kc == 0), stop=(kc == DKV // 128 - 1))
                    gtc = tt * (TW // 128) + tc4
                    nc.vector.tensor_scalar_mul(
                        v_sb[:, :, gtc, 0:128],
                        mm_ps.rearrange("p (h d) -> p h d", h=HL),
                        bcT[tc4])

            # ===== Stage 3: B-Q for all tiles from the gathered cq =====
            # QR (whose rope has a long ACT/DVE tail) runs before UQ so the
            # final tile's tail into the phase-C pool barrier stays short.
            # Dummy Exp: all Sqrts (table set 3) are done, so this pulls the
            # set-0 table reload off phase C's first-exp critical path
            dmy_sb = nrmA.tile([1, 1], f32, tag="dmy")
            nc.scalar.activation(dmy_sb, eps_sb, func=AF.Exp)
            nc.sync.dma_start(
                out=wuq_sb, in_=wuq_d.rearrange("(c p) m -> p c m", p=128))
            nc.sync.dma_start(
                out=wqr_sb, in_=wqr_d.rearrange("(c p) m -> p c m", p=128))
            for c in range(NT):
                cs = slice(c * TW, (c + 1) * TW)
                cq_sb = cqP.tile([128, DQ // 128, TW], f8, tag="cq8")
                nc.gpsimd.dma_start(
                    out=cq_sb,
                    in_=cq_gath[c * DQ:(c + 1) * DQ, :].rearrange(
                        "(c p) t -> p c t", p=128))
                # fp8 DoubleRow: [128, 2, N] APs pair two k-chunks per
                # matmul (256-deep contraction at half the cycles per row)
                DR_MODE = mybir.MatmulPerfMode.DoubleRow
                for j in range(HL // 2):
                    mm_ps = psMM.tile([128, TW], f32, tag="psMM")
                    for kp in range(DQ // 256):
                        nc.tensor.matmul(
                            mm_ps,
                            lhsT=wqr_sb[:, 2 * kp:2 * kp + 2,
                                        j * 128:(j + 1) * 128],
                            rhs=cq_sb[:, 2 * kp:2 * kp + 2, :],
                            start=(kp == 0), stop=(kp == DQ // 256 - 1),
                            perf_mode=DR_MODE)
                    # PE-side rotate_half via the signed permutation, with
                    # scratch tags alternated by tile parity so consecutive
                    # tiles' rope chains don't serialize on buffer WARs
                    if c % 2 == 0:
                        qr_s = sqA.tile([128, TW], bf16, tag="sqA")
                        rot2 = sqA.tile([128, TW], bf16, tag="accKvBf")
                    else:
                        qr_s = sqA.tile([128, TW], bf16, tag="accCq")
                        rot2 = sqA.tile([128, TW], bf16, tag="accCqBf")
                    nc.scalar.copy(qr_s, mm_ps)
                    rot_ps = psBC.tile([128, TW], f32, tag="psBC")
                    nc.tensor.matmul(
                        rot_ps, lhsT=pmat_sb, rhs=qr_s, start=True, stop=True)
                    nc.vector.tensor_mul(qr_s, qr_s, cos_sb[:, cs])
                    nc.vector.tensor_mul(rot2, rot_ps, sin_sb[:, cs])
                    nc.vector.tensor_add(qrT_sb[:, j, cs], qr_s, rot2)
                for h in range(HL):
                    mm_ps = psMM.tile([128, TW], f32, tag="psMM")
                    for kp in range(DQ // 256):
                        nc.tensor.matmul(
                            mm_ps,
                            lhsT=wuq_sb[:, 2 * kp:2 * kp + 2,
                                        h * 128:(h + 1) * 128],
                            rhs=cq_sb[:, 2 * kp:2 * kp + 2, :],
                            start=(kp == 0), stop=(kp == DQ // 256 - 1),
                            perf_mode=DR_MODE)
                    # split the psum drains across ACT and DVE: with the
                    # DoubleRow matmuls this tail is the stage-3 bottleneck
                    if h < 2:
                        nc.scalar.copy(qT_sb[:, h, cs], mm_ps)
                    else:
                        nc.vector.tensor_copy(qT_sb[:, h, cs], mm_ps)

        # ================= Phase C =================
        pC = ctx.enter_context(tc.tile_pool(name="persistC", bufs=1))
        aoT_sb = pC.tile([128, HL, T], bf16, tag="aoT")
        # wo prefetch (used in phase D)
        wo_sb = pC.tile([128, HL, D], bf16, tag="wo")
        nc.sync.dma_start(
            out=wo_sb, in_=wo_d.rearrange("(c p) m -> p c m", p=128))
        # static diagonal mask: mask[k, j] = NEG where j < k (query < key)
        mask_sb = pC.tile([128, 128], f32, tag="mask")
        nc.gpsimd.memset(mask_sb, 0.0)
        nc.gpsimd.affine_select(
            out=mask_sb, in_=mask_sb,
            compare_op=mybir.AluOpType.is_ge, fill=NEG,
            base=0, pattern=[[1, 128]], channel_multiplier=-1)

        with tc.tile_pool(name="ptC", bufs=8) as ptC, \
             tc.tile_pool(name="noC", bufs=3) as noC, \
             tc.tile_pool(name="rC", bufs=2) as rC, \
             tc.tile_pool(name="psS", bufs=3, space="PSUM") as psS, \
             tc.tile_pool(name="psPV", bufs=4, space="PSUM") as psPV, \
             tc.tile_pool(name="psTr", bufs=1, space="PSUM") as psTr:

            # (no_sb, h, g) blocks whose PE transpose is deferred one step so
            # the in-order PE never waits on the DVE normalize chain
            pending = []

            def flush_pending():
                no_sb, fh, fg = pending.pop(0)
                tr_ps = psTr.tile([128, 128], bf16, tag="psTr")
                nc.tensor.transpose(tr_ps, no_sb, ident_bf)
                nc.vector.tensor_copy(
                    aoT_sb[:, fh, fg * 128:(fg + 1) * 128], tr_ps)

            for h in range(HL):
                qr_part = slice((h % 2) * 64, (h % 2) * 64 + 64)
                jj = h // 2
                for c in range(NT):
                    q0 = c * TW
                    b0 = q0 // 128          # global index of first block
                    nkt = 4 * c + 4
                    pv_ps = [psPV.tile([128, 132], f32, tag="psPV",
                                       name=f"pv_{h}_{c}_{i}")
                             for i in range(4)]

                    def emit_pv(kt, pt_sb, h=h, q0=q0, b0=b0, pv_ps=pv_ps):
                        qs_lo = max(q0, kt * 128)
                        for g in range(max(b0, kt), b0 + 4):
                            rel = g - b0
                            off = g * 128 - qs_lo
                            nc.tensor.matmul(
                                pv_ps[rel][:, 0:129],
                                lhsT=pt_sb[:, off:off + 128],
                                rhs=v_sb[:, h, kt, 0:129],
                                start=(kt == 0), stop=(kt == g))
                            if kt == g:  # this query block is complete
                                if pending:
                                    flush_pending()
                                r_sb = rC.tile([128, 1], f32, tag="rC")
                                nc.vector.reciprocal(
                                    r_sb, pv_ps[rel][:, 128:129])
                                no_sb = noC.tile([128, 128], bf16, tag="noC")
                                nc.vector.tensor_scalar_mul(
                                    no_sb, pv_ps[rel][:, 0:128], r_sb)
                                pending.append((no_sb, h, g))

                    inflight = []
                    for kt in range(nkt):
                        k0 = kt * 128
                        qs_lo = max(q0, k0)
                        w = q0 + TW - qs_lo
                        s_ps = psS.tile([128, TW], f32, tag="psS")
                        nc.tensor.matmul(
                            s_ps[:, 0:w], lhsT=kT_sb[:, h, k0:k0 + 128],
                            rhs=qT_sb[:, h, qs_lo:q0 + TW],
                            start=True, stop=False)
                        nc.tensor.matmul(
                            s_ps[:, 0:w], lhsT=kr_rope[qr_part, k0:k0 + 128],
                            rhs=qrT_sb[qr_part, jj, qs_lo:q0 + TW],
                            start=False, stop=True)
                        if k0 >= q0:  # diagonal block leads this row
                            nc.vector.tensor_add(
                                s_ps[:, 0:128], s_ps[:, 0:128], mask_sb)
                        pt_sb = ptC.tile([128, TW], bf16, tag="ptC")
                        nc.scalar.activation(
                            pt_sb[:, 0:w], s_ps[:, 0:w], func=AF.Exp,
                            scale=SCALE)
                        inflight.append((kt, pt_sb))
                        if len(inflight) > 5:  # PV trails S/exp by 5 blocks
                            emit_pv(*inflight.pop(0))
                    for item in inflight:
                        emit_pv(*item)
            while pending:
                flush_pending()

        # ================= Phase D =================
        # out DMAs batched 4 dc-blocks at a time: each dma_start costs ~1us
        # of SP sequencer dispatch, and 64 of them saturate it
        with tc.tile_pool(name="oD", bufs=3) as oD, \
             tc.tile_pool(name="psD", bufs=6, space="PSUM") as psD:
            batches = [(nt, dc0, 4) for nt in range(NT)
                       for dc0 in range(0, D // 128, 4)]
            # short final chain: last batch of the last tile emits singly
            batches = batches[:-1] + [(NT - 1, 12, 3), (NT - 1, 15, 1)]
            for nt, dc0, w in batches:
                ns = slice(nt * TW, (nt + 1) * TW)
                o_sb = oD.tile([128, 4, TW], bf16, tag="oD")
                for dj in range(w):
                    dc = dc0 + dj
                    o_ps = psD.tile([128, TW], f32, tag="psD")
                    for hc in range(HL):
                        nc.tensor.matmul(
                            o_ps,
                            lhsT=wo_sb[:, hc, dc * 128:(dc + 1) * 128],
                            rhs=aoT_sb[:, hc, ns],
                            start=(hc == 0), stop=(hc == HL - 1))
                    nc.scalar.copy(o_sb[:, dj, :], o_ps)
                nc.sync.dma_start(
                    out=out_d[dc0 * 128:(dc0 + w) * 128, ns].rearrange(
                        "(c p) t -> p c t", p=128),
                    in_=o_sb[:, 0:w, :])

    nc.finalize()
    return nc


def _rope_tables():
    inv_freq = (1.0 / (ROPE_BASE ** (np.arange(0, DR, 2, dtype=np.float32) / DR))
                ).astype(np.float32)
    t = np.arange(T, dtype=np.float32)
    freqs = np.outer(t, inv_freq).astype(np.float32)        # (T, 32)
    emb = np.concatenate([freqs, freqs], axis=-1)           # (T, 64)
    cos = np.cos(emb).astype(np.float32).T                  # (64, T)
    sin = np.sin(emb).astype(np.float32).T
    cos128 = np.ascontiguousarray(np.concatenate([cos, cos], 0))  # (128, T)
    sin128 = np.ascontiguousarray(np.concatenate([sin, sin], 0))
    return cos128, sin128


def kernel(x, W_DQ, W_UQ, W_QR, W_DKV, W_UK, W_UV, W_KR, W_O,
           q_norm_w, kv_norm_w):
    global LAST_EXEC_NS
    import ml_dtypes
    from concourse.bass_utils import run_bass_kernel_spmd

    bf = ml_dtypes.bfloat16
    f8 = ml_dtypes.float8_e4m3
    x = np.asarray(x, dtype=np.float32)
    W_DQ = np.asarray(W_DQ, np.float32)
    W_UQ = np.asarray(W_UQ, np.float32)
    W_QR = np.asarray(W_QR, np.float32)
    W_DKV = np.asarray(W_DKV, np.float32)
    W_UK = np.asarray(W_UK, np.float32)
    W_UV = np.asarray(W_UV, np.float32)
    W_KR = np.asarray(W_KR, np.float32)
    W_O = np.asarray(W_O, np.float32)
    q_norm_w = np.asarray(q_norm_w, np.float32)
    kv_norm_w = np.asarray(kv_norm_w, np.float32)

    # fold norm weights into the up-projections (w==1 in practice)
    wuq_f = W_UQ * q_norm_w[:, None]
    wqr_f = W_QR * q_norm_w[:, None]
    wuk_f = W_UK * kv_norm_w[:, None]
    wuv_f = W_UV * kv_norm_w[:, None]

    wall = np.concatenate([W_DQ, W_DKV, W_KR, W_KR], axis=1)
    # (D, MTOT) -> (128, NMC, D): partition-major, latent-block, contraction
    wall = (wall.reshape(NKC, 128, NMC, 128).transpose(1, 2, 0, 3)
            .reshape(128, NMC, D))
    wallq = np.ascontiguousarray(wall[:, 0:8]).astype(bf)
    wallkv = np.ascontiguousarray(wall[:, 8:13]).astype(bf)
    cos128, sin128 = _rope_tables()
    cos128 = cos128.astype(bf)
    sin128 = sin128.astype(bf)

    wuq_h = wuq_f.reshape(DQ, H, DH)
    wqr_h = wqr_f.reshape(DQ, H, DR)
    wuk_h = wuk_f.reshape(DKV, H, DH)
    wuv_h = wuv_f.reshape(DKV, H, DH)
    wo_h = W_O.reshape(H, DH, D)

    in_maps = []
    for ci in range(NCORES):
        b, hg = divmod(ci, H // HL)
        hsl = slice(hg * HL, (hg + 1) * HL)
        xt_b = np.ascontiguousarray(x[b].T).astype(bf)
        in_maps.append({
            "xt": xt_b,
            "xcq": np.ascontiguousarray(xt_b[:, hg * TW:(hg + 1) * TW]),
            "wallq": wallq,
            "wallkv": wallkv,
            "wuq": np.ascontiguousarray(
                wuq_h[:, hsl].reshape(DQ, HL * DH)).astype(f8),
            "wqr": np.ascontiguousarray(
                wqr_h[:, hsl].reshape(DQ, HL * DR)).astype(f8),
            "wuk": np.ascontiguousarray(
                wuk_h[:, hsl].reshape(DKV, HL * DH)).astype(bf),
            "wuv": np.ascontiguousarray(
                wuv_h[:, hsl].reshape(DKV, HL * DH)).astype(bf),
            "wo": np.ascontiguousarray(
                wo_h[hsl].reshape(HL * DH, D)).astype(bf),
            "costab": cos128,
            "sintab": sin128,
        })

    if "nc" not in _CACHE:
        _CACHE["nc"] = _build()
    nc = _CACHE["nc"]

    import os as _os
    _trace = _os.environ.get("MLA_TRACE") == "1"
    res = run_bass_kernel_spmd(
        nc, in_maps, core_ids=list(range(NCORES)), trace=_trace)
    LAST_EXEC_NS = res.exec_time_ns
    outs = [res.results[ci]["final_t"] for ci in range(NCORES)]

    out = np.zeros((B, T, D), np.float32)
    for ci in range(NCORES):
        b = ci // (H // HL)
        out[b] += outs[ci].T.astype(np.float32)
    return out
